# revision 21
# baseline (speedup 1.0000x reference)
"""3-layer GCN (DGI) forward on 8 Trainium2 NeuronCores.

The normalized propagation S = D^-1/2 (A+I) D^-1/2 is applied as a dense
block matmul.  The adjacency (edge multiplicity + self loop, small ints,
fp8-exact) is the 512-wide *moving* operand; the transformed features Z are
the stationary operand.  Each accumulation chain produces h^T[f, t] directly
in PSUM, which is exactly the layout the next layer's XW matmul consumes, so
no transposes are needed between layers.

Normalization folding (biases are zero, prelu is positively homogeneous):
    p_l   = prelu(A' z_l)            (pure activation on the psum chains)
    z_1   = (dinv*X) @ W1            (X pre-scaled on host)
    z_l+1 = dinv^2 * (p_l @ W_l+1)   (per-partition scale in the XW epilogue)
    h_l   = dinv * p_l               (folded into readout mask / score scale)

Sharding: target nodes are sharded 8 ways, 1280 per core (N padded
10000 -> 10240 with isolated phantom nodes).  seq1/seq2 share A and weights.
Layer 1 computes Z1 for all nodes from the replicated inputs (no
communication).  Layers 2-3 AllGather Z per sequence (bf16, 320KB/rank);
each AllGather is issued right after the producing sequence's epilogue and
flies while the other sequence's ~40us of matmul chains execute, so the
collectives are hidden.
"""

import numpy as np
import ml_dtypes

import concourse.bass as bass
import concourse.bacc as bacc
import concourse.mybir as mybir
import concourse.tile as tile
from concourse import bass_utils

BF16 = ml_dtypes.bfloat16
FP8 = ml_dtypes.float8_e4m3

N = 10000          # real nodes
C = 8              # cores
T = 1280           # nodes per core (padded)
NP = C * T         # padded node count 10240
NBT = T // 128     # target blocks per core (10)
NBS = NP // 128    # source blocks (80)
D = 128            # feature dim per sequence
TCH = (512, 512, 256)   # moving-width chunks covering T=1280 targets

_prog_cache = {}


def _build_program(a_prelu: float, b_bilin: float, has_bias: bool, opts=None):
    opts = opts or {}
    n_layers = opts.get("layers", 3)
    use_ag = opts.get("ag", True)
    use_amm = opts.get("amm", True)
    use_readout = opts.get("readout", True) and n_layers > 0
    minimal = opts.get("minimal", False)
    n_reps = opts.get("reps", 1)
    assert not has_bias, "zero-bias fast path only"
    f32 = mybir.dt.float32
    bf16 = mybir.dt.bfloat16
    fp8 = mybir.dt.float8e4
    AF = mybir.ActivationFunctionType

    nc = bacc.Bacc("TRN2", target_bir_lowering=False, debug=False, num_devices=C)

    if minimal:
        out_d = nc.dram_tensor("out", [128, 2 * NBT], f32, kind="ExternalOutput")
        with tile.TileContext(nc) as tc:
            with tc.tile_pool(name="sb", bufs=1) as sb:
                out_sb = sb.tile([128, 2 * NBT], f32, tag="out")
                nc.vector.memset(out_sb[:], 0.0)
                nc.sync.dma_start(out_d[:, :], out_sb[:])
        nc.compile()
        return nc

    n_agonly = opts.get("agonly", 0)      # collectives per rep, no compute
    n_aronly = opts.get("aronly", 0)
    if n_agonly or n_aronly:
        W_Z = NBT * 128
        out_d = nc.dram_tensor("out", [128, 2 * NBT], f32, kind="ExternalOutput")
        agi = nc.dram_tensor("agi", [128, W_Z], mybir.dt.bfloat16)
        ago = nc.dram_tensor("ago", [C * 128, W_Z], mybir.dt.bfloat16,
                             addr_space="Shared")
        ari = nc.dram_tensor("ari", [128, 1], f32)
        aro = nc.dram_tensor("aro", [128, 1], f32, addr_space="Shared")
        rg = [list(range(C))]
        with tile.TileContext(nc) as tc:
            with tc.tile_pool(name="sb", bufs=2) as sb:
                z_sb = sb.tile([128, W_Z], mybir.dt.bfloat16, tag="z", bufs=1)
                nc.vector.memset(z_sb[:], 0.0)
                out_sb = sb.tile([128, 2 * NBT], f32, tag="out", bufs=1)
                nc.vector.memset(out_sb[:], 0.0)
                for rep in range(n_reps):
                    for i in range(n_agonly):
                        zf = sb.tile([128, NBS * 128], mybir.dt.bfloat16,
                                     tag="zf", bufs=1, name="zf")
                        nc.sync.dma_start(agi[:, :], z_sb[:])
                        nc.gpsimd.collective_compute(
                            "AllGather", mybir.AluOpType.bypass,
                            replica_groups=rg,
                            ins=[agi.ap().opt()], outs=[ago.ap().opt()])
                        for r in range(C):
                            nc.sync.dma_start(
                                zf[:, r * W_Z:(r + 1) * W_Z],
                                ago[r * 128:(r + 1) * 128, :])
                        nc.vector.tensor_copy(z_sb[:, 0:1], zf[:, 0:1])
                    for i in range(n_aronly):
                        cs_sb = sb.tile([128, 1], f32, tag="cs", name="cs")
                        nc.vector.memset(cs_sb[:], float(rep))
                        nc.sync.dma_start(ari[:, :], cs_sb[:])
                        nc.gpsimd.collective_compute(
                            "AllReduce", mybir.AluOpType.add,
                            replica_groups=rg,
                            ins=[ari.ap().opt()], outs=[aro.ap().opt()])
                        csum = sb.tile([128, 1], f32, tag="csum", name="csum")
                        nc.sync.dma_start(csum[:], aro[:, :])
                        nc.vector.tensor_copy(z_sb[:, 1:2], csum[:])
                nc.sync.dma_start(out_d[:, :], out_sb[:])
        nc.compile()
        return nc

    At_d = nc.dram_tensor("At", [128, NBS * T], fp8, kind="ExternalInput")
    XTf1_d = nc.dram_tensor("XTf1", [128, NP], bf16, kind="ExternalInput")
    XTf2_d = nc.dram_tensor("XTf2", [128, NP], bf16, kind="ExternalInput")
    dinv2_d = nc.dram_tensor("dinv2", [128, NBT], f32, kind="ExternalInput")
    dinv_d = nc.dram_tensor("dinv", [128, NBT], f32, kind="ExternalInput")
    mkdv_d = nc.dram_tensor("mkdv", [128, NBT], bf16, kind="ExternalInput")
    W_d = nc.dram_tensor("W", [3, 128, 128], bf16, kind="ExternalInput")
    WbT_d = nc.dram_tensor("WbT", [128, 128], f32, kind="ExternalInput")
    ident_d = nc.dram_tensor("ident", [128, 128], bf16, kind="ExternalInput")
    out_d = nc.dram_tensor("out", [128, 2 * NBT], f32, kind="ExternalOutput")

    W_Z = NBT * 128    # z shard row length (1280)
    ag_in = {}
    ag_out = {}
    for l in range(1, 3):
        for s in range(2):
            ag_in[(l, s)] = nc.dram_tensor(f"agin{l}_{s}", [128, W_Z], bf16)
            ag_out[(l, s)] = nc.dram_tensor(
                f"agout{l}_{s}", [C * 128, W_Z], bf16, addr_space="Shared")
    ar_in = nc.dram_tensor("arin", [128, 1], f32)
    ar_out = nc.dram_tensor("arout", [128, 1], f32, addr_space="Shared")
    rg = [list(range(C))]

    with tile.TileContext(nc) as tc:
        with (
            tc.tile_pool(name="sb", bufs=2) as sb,
            tc.tile_pool(name="stat", bufs=1) as stat,
            tc.tile_pool(name="psS", bufs=1, space="PSUM") as psS,
            tc.tile_pool(name="psU", bufs=(4 if opts.get("psu4") else 2), space="PSUM") as psU,
            tc.tile_pool(name="psT", bufs=1, space="PSUM") as psT,
        ):
            # ---- static tiles (X chunks first so layer-1 XW starts early;
            #      At streamed in chain-consumption order behind them) ----
            dinv_sb = stat.tile([128, NBT], f32, tag="dinv")
            nc.sync.dma_start(dinv_sb[:], dinv_d[:, :])
            dinv2_sb = stat.tile([128, NBT], f32, tag="dinv2")
            nc.sync.dma_start(dinv2_sb[:], dinv2_d[:, :])
            mkdv_sb = stat.tile([128, NBT], bf16, tag="mkdv")
            nc.sync.dma_start(mkdv_sb[:], mkdv_d[:, :])
            W_sb = stat.tile([128, 3 * 128], bf16, tag="W")
            for l in range(3):
                nc.sync.dma_start(W_sb[:, l * 128:(l + 1) * 128], W_d[l, :, :])
            WbT_sb = stat.tile([128, 128], f32, tag="WbT")
            nc.sync.dma_start(WbT_sb[:], WbT_d[:, :])
            ident_sb = stat.tile([128, 128], bf16, tag="ident")
            nc.sync.dma_start(ident_sb[:], ident_d[:, :])

            # At on the Activation hwdge queue so the X chunks (SP queue)
            # aren't stuck behind 13MB and layer-1 XW starts immediately
            at_all = stat.tile([128, NBS * T], fp8, tag="at_all")
            n_at_chunks = 16
            for cb in range(n_at_chunks):
                w = NBS * T // n_at_chunks
                nc.scalar.dma_start(at_all[:, cb * w:(cb + 1) * w],
                                    At_d[:, cb * w:(cb + 1) * w])

            use_xstat = opts.get("xstat", True)
            if use_xstat:
                xst = [stat.tile([128, NP], bf16, tag=f"xst{s}",
                                 name=f"xst{s}") for s in range(2)]
                for s, xd in enumerate((XTf1_d, XTf2_d)):
                    for cb in range(C):
                        nc.sync.dma_start(xst[s][:, cb * T:(cb + 1) * T],
                                          xd[:, cb * T:(cb + 1) * T])

            n_chainsrep = opts.get("chainsrep", 0)
            if n_chainsrep:
                zf = [sb.tile([128, NBS * 128], bf16, tag=f"zf{s}", bufs=1,
                              name=f"zf{s}")
                      for s in range(2)]
                for sbk in range(NBS):
                    u2 = psU.tile([128, 256], f32, tag="u", name="u2")
                    for s in range(2):
                        nc.tensor.matmul(
                            u2[:, s * 128:(s + 1) * 128],
                            xst[s][:, sbk * 128:(sbk + 1) * 128],
                            W_sb[:, 0:128], start=True, stop=True)
                    for s in range(2):
                        zslc = zf[s][:, sbk * 128:(sbk + 1) * 128]
                        if sbk % 2 == 0:
                            nc.scalar.copy(zslc, u2[:, s * 128:(s + 1) * 128])
                        else:
                            nc.vector.tensor_copy(
                                zslc, u2[:, s * 128:(s + 1) * 128])
                for rep in range(n_reps):
                    for li in range(n_chainsrep):
                        pT_new = [sb.tile([128, T], bf16, tag=f"pT{s}",
                                          name=f"pTc{s}")
                                  for s in range(2)]
                        for s in range(2):
                            ps = [psS.tile([128, w], f32,
                                           tag=("sA" if w == 512 else "sB"),
                                           bufs=(2 if w == 512 else 1),
                                           name=f"psc{s}{i}")
                                  for i, w in enumerate(TCH)]
                            for sbk in range(NBS):
                                off = 0
                                for i, w in enumerate(TCH):
                                    nc.tensor.matmul(
                                        ps[i][:],
                                        zf[s][:, sbk * 128:(sbk + 1) * 128],
                                        at_all[:, sbk * T + off:sbk * T + off + w],
                                        start=(sbk == 0), stop=(sbk == NBS - 1))
                                    off += w
                            off = 0
                            for i, w in enumerate(TCH):
                                nc.scalar.activation(
                                    pT_new[s][:, off:off + w], ps[i][:],
                                    AF.Prelu, alpha=a_prelu)
                                off += w
                out_sb = sb.tile([128, 2 * NBT], f32, tag="out", name="o")
                nc.vector.memset(out_sb[:], 0.0)
                nc.sync.dma_start(out_d[:, :], out_sb[:])
                raise_done = True
            else:
                raise_done = False

            for rep in range(n_reps if not raise_done else 0):
                # ---- layer 1 XW: z1 for ALL nodes (replicated compute).
                # Issued interleaved with the layer-1 chains below so the PE
                # never idles on the psum->sbuf copies. ----
                zf = [sb.tile([128, NBS * 128], bf16, tag=f"zf{s}", bufs=1,
                              name=f"zf{s}")
                      for s in range(2)]

                def l1_xw(sbk):
                    cb, nb = divmod(sbk, NBT)
                    if use_xstat:
                        def xop(s):
                            return xst[s][:, cb * T + nb * 128:
                                          cb * T + (nb + 1) * 128]
                    else:
                        if nb == 0:
                            xcs = [sb.tile([128, T], bf16, tag=f"xc{s}",
                                           name=f"xc{s}")
                                   for s in range(2)]
                            nc.sync.dma_start(
                                xcs[0][:], XTf1_d[:, cb * T:(cb + 1) * T])
                            nc.sync.dma_start(
                                xcs[1][:], XTf2_d[:, cb * T:(cb + 1) * T])
                            l1_xw.xc = xcs

                        def xop(s):
                            return l1_xw.xc[s][:, nb * 128:(nb + 1) * 128]
                    u2 = psU.tile([128, 256], f32, tag="u", name="u2")
                    for s in range(2):
                        nc.tensor.matmul(
                            u2[:, s * 128:(s + 1) * 128], xop(s),
                            W_sb[:, 0:128], start=True, stop=True)
                    for s in range(2):
                        zslc = zf[s][:, sbk * 128:(sbk + 1) * 128]
                        if sbk % 2 == 0:
                            nc.scalar.copy(zslc, u2[:, s * 128:(s + 1) * 128])
                        else:
                            nc.vector.tensor_copy(
                                zslc, u2[:, s * 128:(s + 1) * 128])

                pT = None
                for l in range(n_layers):
                    pT_new = [
                        sb.tile([128, T], bf16, tag=f"pT{s}", name=f"pT{s}_{l}")
                        for s in range(2)
                    ]
                    for s in range(2):
                        # ---- S @ Z chains: psum accumulates h^T[f, t] ----
                        ps = [psS.tile([128, w], f32,
                                       tag=("sA" if w == 512 else "sB"),
                                       bufs=(2 if w == 512 else 1),
                                       name=f"ps{s}{i}")
                              for i, w in enumerate(TCH)]
                        if use_amm:
                            # the last source block is all phantom nodes, its
                            # A^T panel is exactly zero -> skip it
                            NBSr = NBS - 1
                            for sbk in range(NBSr):
                                if l == 0 and s == 0:
                                    # keep 2 sbk of XW ahead of the chains
                                    if sbk == 0:
                                        l1_xw(0)
                                        l1_xw(1)
                                    if sbk + 2 < NBS:
                                        l1_xw(sbk + 2)
                                    if sbk + 2 == NBSr:
                                        l1_xw(NBS - 1)
                                off = 0
                                for i, w in enumerate(TCH):
                                    nc.tensor.matmul(
                                        ps[i][:],
                                        zf[s][:, sbk * 128:(sbk + 1) * 128],
                                        at_all[:, sbk * T + off:sbk * T + off + w],
                                        start=(sbk == 0), stop=(sbk == NBSr - 1))
                                    off += w
                        else:
                            if l == 0 and s == 0:
                                for sbk in range(NBS):
                                    l1_xw(sbk)
                            for i, w in enumerate(TCH):
                                nc.tensor.matmul(
                                    ps[i][:], zf[s][:, 0:128],
                                    at_all[:, 0:w], start=True, stop=True)
                        # ---- p = prelu(psum), stays transposed [f, t] ----
                        off = 0
                        for i, w in enumerate(TCH):
                            nc.scalar.activation(
                                pT_new[s][:, off:off + w], ps[i][:],
                                AF.Prelu, alpha=a_prelu)
                            off += w

                        if l < n_layers - 1 and opts.get("xwz", True):
                            # ---- z_{l+1} = dinv^2 * (p @ W_{l+1}) ----
                            z_sb = sb.tile([128, W_Z], bf16, tag=f"z{s}",
                                           name=f"z{s}_{l}")
                            for tb in range(NBT):
                                u_ps = psU.tile([128, 128], f32, tag="u",
                                                name="u_ps")
                                nc.tensor.matmul(
                                    u_ps[:],
                                    pT_new[s][:, tb * 128:(tb + 1) * 128],
                                    W_sb[:, (l + 1) * 128:(l + 2) * 128],
                                    start=True, stop=True)
                                nc.scalar.activation(
                                    z_sb[:, tb * 128:(tb + 1) * 128],
                                    u_ps[:], AF.Copy,
                                    scale=dinv2_sb[:, tb:tb + 1])
                            zf_new = sb.tile([128, NBS * 128], bf16, bufs=1,
                                             tag=f"zf{s}", name=f"zf{s}_{l}")
                            if use_ag:
                                nc.sync.dma_start(ag_in[(l + 1, s)][:, :],
                                                  z_sb[:])
                                nc.gpsimd.collective_compute(
                                    "AllGather", mybir.AluOpType.bypass,
                                    replica_groups=rg,
                                    ins=[ag_in[(l + 1, s)].ap().opt()],
                                    outs=[ag_out[(l + 1, s)].ap().opt()])
                                for r in range(C):
                                    nc.sync.dma_start(
                                        zf_new[:, r * W_Z:(r + 1) * W_Z],
                                        ag_out[(l + 1, s)][r * 128:(r + 1) * 128, :])
                            else:
                                nc.sync.dma_start(zf_new[:, 0:W_Z], z_sb[:])
                            zf[s] = zf_new

                        if l == n_layers - 1 and s == 0 and use_readout:
                            # ---- readout from p1 = pT_new[0]:
                            # cs[f] = sum_t p1T[f,t] * (mask*dinv)[t] ----
                            cs_ps = psU.tile([128, 1], f32, tag="cs", bufs=1)
                            for tb in range(NBT):
                                tr_ps = psT.tile([128, 128], bf16, tag="tr",
                                                 bufs=2, name="tr")
                                nc.tensor.transpose(
                                    tr_ps[:],
                                    pT_new[0][:, tb * 128:(tb + 1) * 128],
                                    ident_sb[:])
                                h_sb = sb.tile([128, 128], bf16, tag="hsb",
                                               name="h_sb")
                                nc.vector.tensor_copy(h_sb[:], tr_ps[:])
                                nc.tensor.matmul(
                                    cs_ps[:], h_sb[:], mkdv_sb[:, tb:tb + 1],
                                    start=(tb == 0), stop=(tb == NBT - 1))
                            cs_sb = sb.tile([128, 1], f32, tag="cssb")
                            nc.vector.tensor_copy(cs_sb[:], cs_ps[:])
                            nc.sync.dma_start(ar_in[:, :], cs_sb[:])
                            nc.gpsimd.collective_compute(
                                "AllReduce", mybir.AluOpType.add,
                                replica_groups=rg,
                                ins=[ar_in.ap().opt()],
                                outs=[ar_out.ap().opt()])
                            csum = sb.tile([128, 1], f32, tag="csum")
                            nc.sync.dma_start(csum[:], ar_out[:, :])
                            c_sb = sb.tile([128, 1], f32, tag="c")
                            nc.scalar.activation(c_sb[:], csum[:], AF.Sigmoid,
                                                 scale=1.0 / N)
                            wc_ps = psU.tile([128, 1], f32, tag="cs", bufs=1)
                            nc.tensor.matmul(wc_ps[:], WbT_sb[:], c_sb[:],
                                             start=True, stop=True)
                            wc_bf = sb.tile([128, 1], bf16, tag="wcbf")
                            nc.vector.tensor_copy(wc_bf[:], wc_ps[:])
                    pT = pT_new

                # ---- scores: sc = dinv * (p3 @ wc) + b_bilin ----
                out_sb = sb.tile([128, 2 * NBT], f32, tag="out", name="o")
                if not use_readout:
                    nc.vector.memset(out_sb[:], 0.0)
                else:
                    for s in range(2):
                        for tb in range(NBT):
                            sc_ps = psU.tile([128, 1], f32, tag="u",
                                             name="sc_ps")
                            nc.tensor.matmul(
                                sc_ps[:], pT[s][:, tb * 128:(tb + 1) * 128],
                                wc_bf[:], start=True, stop=True)
                            nc.scalar.activation(
                                out_sb[:, s * NBT + tb: s * NBT + tb + 1],
                                sc_ps[:], AF.Copy,
                                scale=dinv_sb[:, tb:tb + 1])
                    if b_bilin != 0.0:
                        nc.vector.tensor_scalar_add(out_sb[:], out_sb[:],
                                                    b_bilin)
                nc.sync.dma_start(out_d[:, :], out_sb[:])

    nc.compile()
    return nc


def _prepare_inputs(seq1, seq2, edge_index, W1, b1, W2, b2, W3, b3,
                    a_prelu, W_bilin, b_bilin):
    row = np.asarray(edge_index[0], dtype=np.int64)
    col = np.asarray(edge_index[1], dtype=np.int64)

    deg = np.bincount(col, minlength=N).astype(np.float32) + 1.0
    dinv = (1.0 / np.sqrt(deg)).astype(np.float32)
    dinv_pad = np.zeros(NP, np.float32)
    dinv_pad[:N] = dinv
    maskv = np.zeros(NP, np.float32)
    maskv[:N] = 1.0

    # adjacency with multiplicities + self loops; A[t, s] (small ints, fp8 exact)
    A = np.zeros((NP, NP), dtype=np.float32)
    np.add.at(A, (col, row), 1.0)
    idx = np.arange(N)
    A[idx, idx] += 1.0
    Abf = A.astype(FP8)

    # dinv-scaled, transposed, padded inputs (replicated to every core)
    X1 = np.zeros((NP, D), np.float32)
    X1[:N] = np.asarray(seq1, np.float32) * dinv[:, None]
    X2 = np.zeros((NP, D), np.float32)
    X2[:N] = np.asarray(seq2, np.float32) * dinv[:, None]
    XTf1 = np.ascontiguousarray(X1.T).astype(BF16)
    XTf2 = np.ascontiguousarray(X2.T).astype(BF16)

    Wcat = np.stack([
        np.asarray(W1, np.float32),
        np.asarray(W2, np.float32),
        np.asarray(W3, np.float32),
    ]).astype(BF16)
    has_bias = bool(
        np.any(np.asarray(b1)) or np.any(np.asarray(b2))
        or np.any(np.asarray(b3)))

    WbT = np.ascontiguousarray(np.asarray(W_bilin, np.float32).T)
    ident = np.eye(128, dtype=np.float32).astype(BF16)

    def col_layout(v, dtype):
        # [NP] per-core slice -> [128, NBT] (partition = t_local within block)
        return lambda t0: np.ascontiguousarray(
            v[t0:t0 + T].reshape(NBT, 128).T).astype(dtype)

    dv = col_layout(dinv_pad, np.float32)
    dv2 = col_layout(dinv_pad * dinv_pad, np.float32)
    mkdv = col_layout(maskv * dinv_pad, BF16)

    in_maps = []
    for c in range(C):
        t0 = c * T
        # A^T panels: [s_in, sbk, t_local] so panel sbk is [128, T] at
        # cols sbk*T:(sbk+1)*T, used as 512-wide moving operand
        At_c = np.ascontiguousarray(
            Abf[t0:t0 + T, :].T                     # [NP(s), T(t)]
            .reshape(NBS, 128, T)
            .transpose(1, 0, 2)
        ).reshape(128, NBS * T)
        m = {
            "At": At_c,
            "XTf1": XTf1,
            "XTf2": XTf2,
            "dinv": dv(t0),
            "dinv2": dv2(t0),
            "mkdv": mkdv(t0),
            "W": Wcat,
            "WbT": WbT,
            "ident": ident,
        }
        in_maps.append(m)
    return in_maps, has_bias, float(a_prelu), float(b_bilin)


def _run(in_maps, has_bias, a_prelu, b_bilin, **run_kwargs):
    key = (has_bias, a_prelu, b_bilin)
    if key not in _prog_cache:
        _prog_cache[key] = _build_program(a_prelu, b_bilin, has_bias)
    nc = _prog_cache[key]
    res = None
    for attempt in range(3):
        try:
            res = bass_utils.run_bass_kernel_spmd(
                nc, in_maps, core_ids=list(range(C)), **run_kwargs
            )
            break
        except Exception:
            if attempt == 2:
                raise
            import time
            time.sleep(2.0)
    parts = []
    for c in range(C):
        o = np.asarray(res.results[c]["out"], np.float32)     # [128, 2*NBT]
        parts.append(o.reshape(128, 2, NBT).transpose(1, 2, 0).reshape(2, T))
    sc = np.concatenate(parts, axis=1)                        # [2, NP]
    out = np.concatenate([sc[0, :N], sc[1, :N]]).astype(np.float32)
    return out, res


def kernel(**inputs):
    in_maps, has_bias, a_prelu, b_bilin = _prepare_inputs(**inputs)
    out, _ = _run(in_maps, has_bias, a_prelu, b_bilin)
    return out
